# revision 1
# baseline (speedup 1.0000x reference)
"""Trainium2 Bass kernel for nn_DCTExtractor.

Reference computation:
  - stego [8, 3, 1024, 1024] f32; per 8x8 block 2D DCT-II (D @ X @ D^T).
  - bits[i] = abs(round_half_even(dct[b,c,nh,nw,bh,bw])) % 2 for 1572864
    index tuples.
  - out [8, num_bits]: out[b_idx[i], i] = bits[i]; other rows 0.

Sharding: data-parallel over batch b across the 8 NeuronCores; core b
processes image b and produces output row b.

Fast path (used when the index arrays match the canonical meshgrid pattern
from setup_inputs(), i.e. every (b,c,nh,nw) block contributes positions
(1,2),(2,1),(2,2),(3,1) in row-major (b,c,nh,nw,p) order): the kernel only
computes those 4 coefficients per block with two fp32 matmul stages:

  stage A (contract j, fused transpose):  Y = Xchunk^T @ BR
     lhsT = X strip chunk [128(h=nhl*8+j), 128(w)]  (stationary)
     rhs  = BR [128(h), 48(nhl*3+i')] block-diag D[1+i', j]
     out  = Y [128(w = nwl*8+k), 48(nhl, i')]
  stage B (contract k, one batched matmul per strip):  F = BCp^T @ Y
     lhsT = BCp [128(w=nwl*8+k), 128(p*32+nwl)] block-diag D[l_p, k]
     rhs  = Y [128, 384]  (all 8 w-chunks share the partition dim)
     out  = F [128(p*32+nwl), 384(wc, nhl, i')]  (32-aligned p-planes)

  parity: ACT Abs extraction per plane, then r=RNE via the 2^23 trick,
  parity = 2*|r/2 - RNE(r/2)|.

General fallback (arbitrary indices): device computes the full 64-plane
parity table per image; host gathers bits and applies the b mask.
"""

import sys

if "/opt/trn_rl_repo" not in sys.path:
    sys.path.insert(0, "/opt/trn_rl_repo")

import numpy as np

BS = 8
B, C, H, W = 8, 3, 1024, 1024
NBH, NBW = H // BS, W // BS
POS = np.array([[1, 2], [2, 1], [2, 2], [3, 1]], dtype=np.int32)
NPOS = 4
SEG = C * NBH * NBW * NPOS  # bits per batch element = 196608
NUM_BITS = B * SEG
NSTRIP = C * (H // 128)  # 24 strips of 128 image rows per image
MAGIC = float(np.float32(8388608.0))  # 2^23: a + 2^23 - 2^23 == RNE(a)
IP = [0, 1, 1, 2]  # i' = bh-1 per p
LP = [2, 1, 2, 1]  # l  = bw   per p

_CACHE = {}


def _split_sync_waits(nc):
    """The staged walrus build accepts at most ONE sync wait per
    instruction, but Tile's wait-assignment freely attaches several.
    Hoist all but the last wait of each instruction onto same-engine
    NoOps inserted directly before it (engines execute their stream in
    order, so the semantics are identical)."""
    from concourse import mybir

    if getattr(nc, "_sync_waits_split", False):
        return
    nc._sync_waits_split = True
    counter = 0
    for bb in nc.m.functions[0].blocks:
        out = []
        changed = False
        for inst in bb.instructions:
            si = inst.sync_info
            waits = list(si.on_wait) if si is not None else []
            if len(waits) > 1:
                for w in waits[:-1]:
                    nop = mybir.InstNoOp(
                        name=f"I-splitw-{counter}", ins=[], outs=[])
                    counter += 1
                    nop.engine = inst.engine
                    nop.sync_info = mybir.SyncInfo(on_update=[], on_wait=[w])
                    out.append(nop)
                si.on_wait = waits[-1:]
                changed = True
            out.append(inst)
        if changed:
            bb.instructions = out


def _dct_matrix_f32() -> np.ndarray:
    k = np.arange(BS)[:, None].astype(np.float64)
    m = np.arange(BS)[None, :].astype(np.float64)
    D = np.cos(np.pi * (2.0 * m + 1.0) * k / (2.0 * BS)) * np.sqrt(2.0 / BS)
    D[0, :] = np.sqrt(1.0 / BS)
    return D.astype(np.float32)


def _canonical_indices():
    b, c, nh, nw, p = np.meshgrid(
        np.arange(B), np.arange(C), np.arange(NBH), np.arange(NBW),
        np.arange(NPOS), indexing="ij")
    return {
        "b_idx": b.reshape(-1).astype(np.int32),
        "c_idx": c.reshape(-1).astype(np.int32),
        "nh_idx": nh.reshape(-1).astype(np.int32),
        "nw_idx": nw.reshape(-1).astype(np.int32),
        "bh_idx": POS[p.reshape(-1), 0].astype(np.int32),
        "bw_idx": POS[p.reshape(-1), 1].astype(np.int32),
    }


def _is_canonical(b_idx, c_idx, nh_idx, nw_idx, bh_idx, bw_idx) -> bool:
    if b_idx.shape[0] != NUM_BITS:
        return False
    canon = _CACHE.setdefault("canon", _canonical_indices())
    got = {"b_idx": b_idx, "c_idx": c_idx, "nh_idx": nh_idx,
           "nw_idx": nw_idx, "bh_idx": bh_idx, "bw_idx": bw_idx}
    return all(np.array_equal(np.asarray(got[k]), canon[k]) for k in canon)


def _build_consts_fast():
    D = _dct_matrix_f32()
    BR = np.zeros((128, 48), dtype=np.float32)
    for nhl in range(16):
        # block [j, i'] = D[1+i', j]
        BR[nhl * 8:(nhl + 1) * 8, nhl * 3:(nhl + 1) * 3] = D[1:4, :].T
    # plane p lives at partitions p*32..p*32+16 — PSUM reads need the
    # partition offset 32-aligned, so planes are padded to 32 rows.
    BCp = np.zeros((128, 128), dtype=np.float32)
    for p in range(NPOS):
        for nwl in range(16):
            BCp[nwl * 8:(nwl + 1) * 8, p * 32 + nwl] = D[LP[p], :]
    return BR, BCp


def _build_consts_general():
    D = _dct_matrix_f32()
    BR8 = np.zeros((128, 128), dtype=np.float32)
    for nhl in range(16):
        BR8[nhl * 8:(nhl + 1) * 8, nhl * 8:(nhl + 1) * 8] = D.T  # [j, i]
    BC8 = np.zeros((128, 128), dtype=np.float32)
    for l in range(8):
        for nwl in range(16):
            BC8[nwl * 8:(nwl + 1) * 8, l * 16 + nwl] = D[l, :]
    return BR8, BC8


def _parity_ops(nc, pk, hk):
    """pk holds |coeff|. Compute parity = |RNE(pk)| mod 2 into pk using only
    add/sub/mul/abs (this walrus build lacks a DVE mod op):
      r  = (pk + 2^23) - 2^23        round-half-even to integer
      h  = r * 0.5
      rh = (h + 2^23) - 2^23         = h if r even, else nearest even int
      parity = 2*|h - rh|            0.0 or 1.0
    Each step is its own instruction so every intermediate is rounded f32."""
    from concourse import mybir

    ts = nc.vector.tensor_scalar
    add, sub, mult = (mybir.AluOpType.add, mybir.AluOpType.subtract,
                      mybir.AluOpType.mult)
    ts(out=pk[:], in0=pk[:], scalar1=MAGIC, scalar2=None, op0=add)
    ts(out=pk[:], in0=pk[:], scalar1=MAGIC, scalar2=None, op0=sub)
    ts(out=hk[:], in0=pk[:], scalar1=0.5, scalar2=None, op0=mult)
    ts(out=pk[:], in0=hk[:], scalar1=MAGIC, scalar2=None, op0=add)
    ts(out=pk[:], in0=pk[:], scalar1=MAGIC, scalar2=None, op0=sub)
    nc.vector.tensor_tensor(
        out=pk[:], in0=hk[:], in1=pk[:], op=sub)
    nc.scalar.activation(
        out=pk[:], in_=pk[:], func=mybir.ActivationFunctionType.Abs,
        scale=2.0)


def build_fast_nc(nstrip=NSTRIP, mm_dtype=None):
    """Per-core program: x [3,1024,1024] f32 -> o [nstrip, 128, 128] f32.

    o[s=(c,hg), p*32 + nwl, wc*16 + nhl] (nwl < 16; rows 16..31 of each
    32-row plane are dead) = parity of dct coeff (bh=1+IP[p], bw=LP[p])
    of block (c, nh=hg*16+nhl, nw=wc*16+nwl).
    """
    import concourse.bass as bass
    import concourse.tile as tile
    from concourse import mybir

    f32 = mybir.dt.float32
    nc = bass.Bass()
    x = nc.dram_tensor("x", [C, H, W], f32, kind="ExternalInput")
    br = nc.dram_tensor("br", [128, 48], f32, kind="ExternalInput")
    bc = nc.dram_tensor("bc", [128, 128], f32, kind="ExternalInput")
    o = nc.dram_tensor("o", [nstrip, 128, 128], f32, kind="ExternalOutput")

    with tile.TileContext(nc) as tc:
        with (
            tc.tile_pool(name="consts", bufs=1) as consts,
            tc.tile_pool(name="xs", bufs=6) as xpool,
            tc.tile_pool(name="ysb", bufs=3) as ypool,
            tc.tile_pool(name="pk", bufs=4) as pkpool,
            tc.tile_pool(name="yp", bufs=4, space="PSUM") as yppool,
            tc.tile_pool(name="fp", bufs=3, space="PSUM") as fppool,
        ):
            brt = consts.tile([128, 48], f32)
            nc.sync.dma_start(out=brt[:], in_=br[:, :])
            bct = consts.tile([128, 128], f32)
            nc.sync.dma_start(out=bct[:], in_=bc[:, :])

            def cast(ap):
                return ap.bitcast(mm_dtype) if mm_dtype is not None else ap

            for s in range(nstrip):
                c, hg = divmod(s, H // 128)
                xs = xpool.tile([128, 1024], f32, tag="xs")
                nc.sync.dma_start(
                    out=xs[:], in_=x[c, hg * 128:(hg + 1) * 128, :])
                # stage A: 8 chunk matmuls into one PSUM tile, one copy out
                yp = yppool.tile([128, 384], f32, tag="yp")
                for wc in range(8):
                    nc.tensor.matmul(
                        out=yp[:, wc * 48:(wc + 1) * 48],
                        lhsT=cast(xs[:, wc * 128:(wc + 1) * 128]),
                        rhs=cast(brt[:]),
                        start=True, stop=True)
                ysb = ypool.tile([128, 384], f32, tag="ysb")
                nc.vector.tensor_copy(out=ysb[:], in_=yp[:])
                # stage B: one batched matmul (chunks share the partition dim)
                fp = fppool.tile([128, 384], f32, tag="fp")
                nc.tensor.matmul(
                    out=fp[:],
                    lhsT=cast(bct[:]),
                    rhs=cast(ysb[:]),
                    start=True, stop=True)
                pk = pkpool.tile([128, 128], f32, tag="pk")
                hk = pkpool.tile([128, 128], f32, tag="hk")
                nc.gpsimd.memset(pk[:], 0.0)
                for p in range(NPOS):
                    # plane p lives in partitions p*32..p*32+16; take i'=IP[p]
                    src = fp[p * 32:p * 32 + 16, :].rearrange(
                        "q (wc nhl i) -> q wc nhl i", nhl=16, i=3)[:, :, :, IP[p]]
                    dst = pk[p * 32:p * 32 + 16, :].rearrange(
                        "q (wc nhl) -> q wc nhl", nhl=16)
                    nc.scalar.activation(
                        out=dst, in_=src,
                        func=mybir.ActivationFunctionType.Abs)
                # parity over the full tile; rows 16..31 of each 32-block are
                # dead lanes (whatever the slot held) and are skipped by the
                # output DMA below.
                _parity_ops(nc, pk, hk)
                # one big DMA per strip; dead rows (16..31 of each 32-block)
                # ride along and are sliced off on the host.
                nc.sync.dma_start(out=o[s], in_=pk[:])
    return nc


def build_general_nc(nstrip=NSTRIP):
    """Per-core program: full 64-plane parity table.

    table [nstrip, 128, 1024] f32 where
    table[s=(c,hg), l*16+nwl, wc*128 + nhl*8 + i] =
        parity of dct coeff (bh=i, bw=l) of block (c, hg*16+nhl, wc*16+nwl).
    """
    import concourse.bass as bass
    import concourse.tile as tile
    from concourse import mybir

    f32 = mybir.dt.float32
    nc = bass.Bass()
    x = nc.dram_tensor("x", [C, H, W], f32, kind="ExternalInput")
    br = nc.dram_tensor("br", [128, 128], f32, kind="ExternalInput")
    bc = nc.dram_tensor("bc", [128, 128], f32, kind="ExternalInput")
    o = nc.dram_tensor("o", [nstrip, 128, 1024], f32, kind="ExternalOutput")

    with tile.TileContext(nc) as tc:
        with (
            tc.tile_pool(name="consts", bufs=1) as consts,
            tc.tile_pool(name="xs", bufs=2) as xpool,
            tc.tile_pool(name="ysb", bufs=2) as ypool,
            tc.tile_pool(name="pk", bufs=2) as pkpool,
            tc.tile_pool(name="yp", bufs=4, space="PSUM") as yppool,
            tc.tile_pool(name="fp", bufs=4, space="PSUM") as fppool,
        ):
            brt = consts.tile([128, 128], f32)
            nc.sync.dma_start(out=brt[:], in_=br[:, :])
            bct = consts.tile([128, 128], f32)
            nc.sync.dma_start(out=bct[:], in_=bc[:, :])

            for s in range(nstrip):
                c, hg = divmod(s, H // 128)
                xs = xpool.tile([128, 1024], f32, tag="xs")
                nc.sync.dma_start(
                    out=xs[:], in_=x[c, hg * 128:(hg + 1) * 128, :])
                ysb = ypool.tile([128, 1024], f32, tag="ysb")
                for wc in range(8):
                    yp = yppool.tile([128, 128], f32, tag="yp")
                    nc.tensor.matmul(
                        out=yp[:],
                        lhsT=xs[:, wc * 128:(wc + 1) * 128],
                        rhs=brt[:],
                        start=True, stop=True)
                    nc.vector.tensor_copy(
                        out=ysb[:, wc * 128:(wc + 1) * 128], in_=yp[:])
                pk = pkpool.tile([128, 1024], f32, tag="pk")
                hk = pkpool.tile([128, 1024], f32, tag="hk")
                for wc in range(8):
                    fp = fppool.tile([128, 128], f32, tag="fp")
                    nc.tensor.matmul(
                        out=fp[:],
                        lhsT=bct[:],
                        rhs=ysb[:, wc * 128:(wc + 1) * 128],
                        start=True, stop=True)
                    nc.scalar.activation(
                        out=pk[:, wc * 128:(wc + 1) * 128], in_=fp[:],
                        func=mybir.ActivationFunctionType.Abs)
                _parity_ops(nc, pk, hk)
                nc.sync.dma_start(out=o[s], in_=pk[:])
    return nc


def _run_spmd(nc, in_maps, trace=False):
    from concourse.bass_utils import run_bass_kernel_spmd

    _split_sync_waits(nc)

    res = run_bass_kernel_spmd(
        nc, in_maps, core_ids=list(range(B)), trace=trace)
    _CACHE["last_results"] = res
    return res.results


def _fast_path(stego, trace=False):
    key = "fast_nc"
    if key not in _CACHE:
        _CACHE[key] = build_fast_nc()
    nc = _CACHE[key]
    BR, BCp = _CACHE.setdefault("consts_fast", _build_consts_fast())
    in_maps = [
        {"x": np.ascontiguousarray(stego[b]), "br": BR, "bc": BCp}
        for b in range(B)
    ]
    results = _run_spmd(nc, in_maps, trace=trace)
    out = np.zeros((B, NUM_BITS), dtype=np.float32)
    for b in range(B):
        O = results[b]["o"]  # [24, 128, 128]; live rows are z<16 of each 32
        Ol = O.reshape(NSTRIP, NPOS, 32, 128)[:, :, :16, :]
        O6 = Ol.reshape(C, 8, NPOS, 16, 8, 16)  # c, hg, p, nwl, wc, nhl
        seg = np.ascontiguousarray(O6.transpose(0, 1, 5, 4, 3, 2)).reshape(-1)
        out[b, b * SEG:(b + 1) * SEG] = seg
    return out


def _general_path(stego, b_idx, c_idx, nh_idx, nw_idx, bh_idx, bw_idx,
                  trace=False):
    key = "general_nc"
    if key not in _CACHE:
        _CACHE[key] = build_general_nc()
    nc = _CACHE[key]
    BR8, BC8 = _CACHE.setdefault("consts_general", _build_consts_general())
    in_maps = [
        {"x": np.ascontiguousarray(stego[b]), "br": BR8, "bc": BC8}
        for b in range(B)
    ]
    results = _run_spmd(nc, in_maps, trace=trace)

    b_idx = np.asarray(b_idx).astype(np.int64)
    c_idx = np.asarray(c_idx).astype(np.int64)
    nh_idx = np.asarray(nh_idx).astype(np.int64)
    nw_idx = np.asarray(nw_idx).astype(np.int64)
    bh_idx = np.asarray(bh_idx).astype(np.int64)
    bw_idx = np.asarray(bw_idx).astype(np.int64)
    num_bits = b_idx.shape[0]

    # table[s=(c,hg), l*16+nwl, wc*128 + nhl*8 + i]
    s = c_idx * 8 + nh_idx // 16
    part = bw_idx * 16 + nw_idx % 16
    free = (nw_idx // 16) * 128 + (nh_idx % 16) * 8 + bh_idx
    flat = (s * 128 + part) * 1024 + free

    out = np.zeros((B, num_bits), dtype=np.float32)
    cols = np.arange(num_bits)
    for b in range(B):
        tb = results[b]["o"].reshape(-1)
        mask = b_idx == b
        out[b, cols[mask]] = tb[flat[mask]]
    return out


def kernel(stego, b_idx, c_idx, nh_idx, nw_idx, bh_idx, bw_idx):
    stego = np.ascontiguousarray(np.asarray(stego, dtype=np.float32))
    import os
    trace = os.environ.get("BASS_TRACE", "") not in ("", "0")
    if _is_canonical(b_idx, c_idx, nh_idx, nw_idx, bh_idx, bw_idx):
        return _fast_path(stego, trace=trace)
    return _general_path(
        stego, b_idx, c_idx, nh_idx, nw_idx, bh_idx, bw_idx, trace=trace)

